# revision 6
# baseline (speedup 1.0000x reference)
"""Bass/Trainium2 kernel for nn_CrossWindowAttention3D (8-core SPMD).

Strategy (hardcoded for shapes B_=1024, N=98, C=96, H=3, NW=512):
- Shard 1024 window-instances over 8 cores: core c owns distinct windows
  [64c, 64c+64) for both batch replicas, interleaved (b0,j),(b1,j) so the
  mask+bias table for window j is loaded once per pair.
- Host precomputes channel-major bf16 transposes of x/y, per-head composite
  matrices M_h = scale * Wq_h^T Wk_h (so logits_h = x^T (M_h y) and no
  separate k projection is needed), the additive mask+bias table
  amb[tk, j, h*98+tq] = mask[j, tq, tk] + bias[h, tq, tk], and the proj
  weight with a trailing bias row (pw2 [97, 96], row 96 = proj_b).
- Device, per 4-window group (2 pairs x 2 batch replicas):
  * zt = M_h y for 4 windows (3 full matmuls, 392 cols each)
  * per window: mask+bias accumulated into logits PSUM via an
    identity-stationary matmul, then logits = x^T zt on top (PSUM acc);
    the same x stationary also produces token-major v (v = x^T Wv).
  * one quad-window exp on ACT ([98, 4, 294] spanning 4 PSUM banks)
  * softmax denominators via narrow ones-stationary matmuls (out 32-row
    col-groups run concurrently in the PE array), reciprocal via the
    1-op DVE reciprocal_approx_fast, av via 12 narrow matmuls, normalize
    with one DVE multiply into a [97, 392] tile whose last row is 1.0,
    projection (+bias via the ones row) as one 392-col matmul, output
    staged bf16 and DMA'd channel-major; host transposes/casts to f32.
- PSUM budget: shared 4-bank region (zt slots then logits), pv 1, pd 1,
  pav 1, psy 1 = 8 banks. Emission is software-pipelined: group g+1's zt
  and logits interleave with group g's denominator/av/proj tail.
"""

import sys

sys.path.insert(0, "/opt/trn_rl_repo")

import numpy as np
import ml_dtypes

import concourse.bass as bass
import concourse.tile as tile
from concourse import mybir
from concourse.vector_clock import ScopedClock
from concourse.bass_utils import run_bass_kernel_spmd

BF16 = mybir.dt.bfloat16
F32 = mybir.dt.float32
NPBF16 = ml_dtypes.bfloat16

WS = (2, 7, 7)
N = 98            # tokens per window
C = 96            # embed dim
H = 3             # heads
HD = 32           # head dim
NW = 512          # distinct windows
BWIN = 1024       # window-instances total
NCORES = 8
NI = 128          # instances per core
NJ = 64           # distinct windows per core
T = NI * N        # tokens per core = 12544
HB = H * N        # 294
G = 4             # windows per group
NG = NI // G      # 32 groups


# ---------------------------------------------------------------- tile patch
def _patch_tile_tail_drain():
    """This neuronxcc build rejects >1 sync wait on CTRL-class (Drain)
    instructions; split the TileContext tail-drain waits across NOPs."""
    if getattr(tile.TileContext, "_drain_patch_applied", False):
        return

    def _drain_and_barrier_split(self, tick_clock, wait_clock):
        nc = self.nc
        carrier = nc.sync.nop(nofuse=True)
        wait_clock.add_sem_waits(
            carrier.ins, ScopedClock({None: tick_clock.global_clock})
        )
        si = carrier.ins.sync_info
        waits = list(si.on_wait or []) if si is not None else []
        if len(waits) > 1:
            si.on_wait = waits[:1]
            for w in waits[1:]:
                extra = nc.sync.nop(nofuse=True)
                esi = extra.ins.sync_info
                if esi is None:
                    extra.ins.sync_info = mybir.SyncInfo(
                        on_wait=[w], on_update=[]
                    )
                else:
                    esi.on_wait = list(esi.on_wait or []) + [w]
        nc.sync.drain()
        nc.all_engine_barrier()
        assert self.sems is not None
        popped = nc._tile_sem_poison_stack.pop()
        assert popped is self._sem_poison
        nc.clear_and_free_semaphores(list(self.sems.allocated().values()))
        nc.all_engine_barrier()

    tile.TileContext._drain_and_barrier = _drain_and_barrier_split
    tile.TileContext._drain_patch_applied = True


def _split_sync_waits(nc, max_waits=1):
    """This neuronxcc build accepts at most one sync wait per instruction.
    Hoist excess waits onto same-engine NOPs inserted just before the
    instruction (the sequencer blocks on them in order; AND-semantics of
    multiple waits is preserved)."""
    ctr = 0
    for bb in nc.main_func.blocks:
        new_list = []
        changed = False
        for inst in bb.instructions:
            si = inst.sync_info
            waits = list(si.on_wait or []) if si is not None else []
            if len(waits) > max_waits:
                si.on_wait = waits[: max_waits]
                for w in waits[max_waits:]:
                    nop = mybir.InstNoOp(
                        name=f"I-waitsplit-{ctr}", ins=[], outs=[]
                    )
                    ctr += 1
                    nop.engine = inst.engine
                    nop.sync_info = mybir.SyncInfo(on_wait=[w], on_update=[])
                    new_list.append(nop)
                changed = True
            new_list.append(inst)
        if changed:
            bb.instructions = new_list


# ------------------------------------------------------------- host helpers
def _relative_position_index():
    ws = WS
    coords = np.stack(
        np.meshgrid(
            np.arange(ws[0]), np.arange(ws[1]), np.arange(ws[2]), indexing="ij"
        )
    )
    cf = coords.reshape(3, -1)
    rel = cf[:, :, None] - cf[:, None, :]
    rel = rel.transpose(1, 2, 0).astype(np.int64)
    rel[..., 0] += ws[0] - 1
    rel[..., 1] += ws[1] - 1
    rel[..., 2] += ws[2] - 1
    rel[..., 0] *= (2 * ws[1] - 1) * (2 * ws[2] - 1)
    rel[..., 1] *= 2 * ws[2] - 1
    return rel.sum(-1)  # (N, N)


REL_IDX = _relative_position_index()


# ------------------------------------------------------------ device program
_PROGRAM = None

# tiling knobs
XCH = 32          # instances per x/y SBUF chunk (4 chunks)
ACH = 16          # distinct windows per amb SBUF chunk
YB = 8            # windows per output staging buffer / DMA


def _build_program(split_waits=True):
    _patch_tile_tail_drain()
    nc = bass.Bass()

    xT = nc.declare_dram_parameter("xT", [C, T], BF16, isOutput=False)
    yT = nc.declare_dram_parameter("yT", [C, T], BF16, isOutput=False)
    amb = nc.declare_dram_parameter("amb", [N, NJ, HB], BF16, isOutput=False)
    zw = nc.declare_dram_parameter("zw", [C, H, C], BF16, isOutput=False)
    wv = nc.declare_dram_parameter("wv", [C, C], BF16, isOutput=False)
    ident = nc.declare_dram_parameter("ident", [N, N], BF16, isOutput=False)
    pw2 = nc.declare_dram_parameter("pw2", [C + 1, C], BF16, isOutput=False)
    out = nc.declare_dram_parameter("yT_out", [C, T], BF16, isOutput=True)

    from contextlib import ExitStack

    with tile.TileContext(nc) as tc:
        with ExitStack() as ctx:
            singles = ctx.enter_context(tc.tile_pool(name="singles", bufs=1))
            xt_pool = ctx.enter_context(tc.tile_pool(name="xt", bufs=2))
            yt_pool = ctx.enter_context(tc.tile_pool(name="yt", bufs=2))
            amb_pool = ctx.enter_context(tc.tile_pool(name="amb", bufs=2))
            ztq_pool = ctx.enter_context(tc.tile_pool(name="ztq", bufs=2))
            expT_pool = ctx.enter_context(tc.tile_pool(name="expT", bufs=2))
            v4_pool = ctx.enter_context(tc.tile_pool(name="v4", bufs=2))
            r2_pool = ctx.enter_context(tc.tile_pool(name="r2", bufs=2))
            avT_pool = ctx.enter_context(tc.tile_pool(name="avT", bufs=2))
            ystage_pool = ctx.enter_context(
                tc.tile_pool(name="ystage", bufs=2)
            )
            # PSUM: shared 4-bank region (zt then logits) + pv/pd/pav/psy
            ps_R = ctx.enter_context(
                tc.tile_pool(name="ps_R", bufs=1, space="PSUM")
            )
            ps_v = ctx.enter_context(
                tc.tile_pool(name="ps_v", bufs=1, space="PSUM")
            )
            ps_d = ctx.enter_context(
                tc.tile_pool(name="ps_d", bufs=1, space="PSUM")
            )
            ps_av = ctx.enter_context(
                tc.tile_pool(name="ps_av", bufs=1, space="PSUM")
            )
            ps_y = ctx.enter_context(
                tc.tile_pool(name="ps_y", bufs=1, space="PSUM")
            )

            zw_sb = singles.tile([C, H, C], BF16)
            nc.sync.dma_start(out=zw_sb, in_=zw[:, :, :])
            wv_sb = singles.tile([C, C], BF16)
            nc.sync.dma_start(out=wv_sb, in_=wv[:, :])
            id_sb = singles.tile([N, N], BF16)
            nc.sync.dma_start(out=id_sb, in_=ident[:, :])
            pw2_sb = singles.tile([C + 1, C], BF16)
            nc.sync.dma_start(out=pw2_sb, in_=pw2[:, :])
            ones_sb = singles.tile([N, HD], BF16)
            nc.vector.memset(ones_sb, 1.0)

            # per-group state carried across the pipelined loop
            st = {}
            xt_ch = yt_ch = amb_ch = None
            ystage = None

            def emit_head(g):
                """zt matmuls + cast for group g (uses shared region R)."""
                nonlocal xt_ch, yt_ch, amb_ch
                w0 = G * g
                if w0 % XCH == 0:
                    ch = w0 // XCH
                    xt_ch = xt_pool.tile([C, XCH * N], BF16)
                    nc.sync.dma_start(
                        out=xt_ch,
                        in_=xT[:, ch * XCH * N : (ch + 1) * XCH * N],
                    )
                    yt_ch = yt_pool.tile([C, XCH * N], BF16)
                    nc.sync.dma_start(
                        out=yt_ch,
                        in_=yT[:, ch * XCH * N : (ch + 1) * XCH * N],
                    )
                # distinct windows for group g are 2g, 2g+1
                if (2 * g) % ACH == 0:
                    ak = (2 * g) // ACH
                    amb_ch = amb_pool.tile([N, ACH, HB], BF16)
                    nc.sync.dma_start(
                        out=amb_ch, in_=amb[:, ak * ACH : (ak + 1) * ACH, :]
                    )

                R = ps_R.tile([N, G, 512], F32)
                goff = (w0 % XCH) * N
                # zt: 3 full matmuls into slots 0..2 rows 0..95
                for h in range(H):
                    nc.tensor.matmul(
                        out=R[0:C, h, 0 : G * N],
                        lhsT=zw_sb[:, h, :],
                        rhs=yt_ch[:, goff : goff + G * N],
                    )
                ztq = ztq_pool.tile([C, H, G * N], BF16)
                nc.vector.tensor_copy(ztq, R[0:C, 0:H, 0 : G * N])
                st["R"] = R
                st["ztq"] = ztq
                st["goff"] = goff
                st["amb_ch"] = amb_ch
                st["xt_ch"] = xt_ch

            def emit_logits(g):
                """mask+bias accumulate, logits, v, exp, vcopy for group g."""
                R = st["R"]
                ztq = st["ztq"]
                goff = st["goff"]
                a_ch = st["amb_ch"]
                x_ch = st["xt_ch"]
                pv = ps_v.tile([N, G, 128], F32)
                for pr in range(2):
                    j = 2 * g + pr           # distinct window
                    aj = j % ACH
                    for k in range(2):
                        w = 2 * pr + k       # window slot in group
                        # mask+bias into PSUM (identity stationary)
                        nc.tensor.matmul(
                            out=R[:, w, 0:HB],
                            lhsT=id_sb,
                            rhs=a_ch[:, aj, :],
                            start=True,
                            stop=False,
                        )
                        # logits on top; x stationary shared with v
                        col = goff + w * N
                        nc.tensor.matmul(
                            out=R[:, w, 0:HB],
                            lhsT=x_ch[:, col : col + N],
                            rhs=ztq[:, :, w * N : (w + 1) * N],
                            start=False,
                            stop=True,
                        )
                        nc.tensor.matmul(
                            out=pv[:, w, 0:C],
                            lhsT=x_ch[:, col : col + N],
                            rhs=wv_sb,
                        )
                expT = expT_pool.tile([N, G, HB], BF16)
                nc.scalar.activation(
                    out=expT,
                    in_=R[:, :, 0:HB],
                    func=mybir.ActivationFunctionType.Exp,
                )
                v4 = v4_pool.tile([N, G, C], BF16)
                nc.vector.tensor_copy(v4, pv[:, :, 0:C])
                st["expT"] = expT
                st["v4"] = v4

            def emit_tail(g):
                """denominators, reciprocal, av, normalize, proj, out."""
                nonlocal ystage
                expT = st["expT"]
                v4 = st["v4"]
                pd = ps_d.tile([C, G, N], F32)
                for h in range(H):
                    nc.tensor.matmul(
                        out=pd[h * HD : (h + 1) * HD, :, :],
                        lhsT=ones_sb,
                        rhs=expT[:, :, h * N : (h + 1) * N],
                    )
                # 1/d = exp(-ln(d)); Ln+Exp share one ACT table set
                t_ln = r2_pool.tile([C, G, N], F32, name="t_ln", tag="tl")
                nc.scalar.activation(
                    out=t_ln,
                    in_=pd,
                    func=mybir.ActivationFunctionType.Ln,
                )
                r2 = r2_pool.tile([C, G, N], F32, name="r2", tag="r2")
                nc.scalar.activation(
                    out=r2,
                    in_=t_ln,
                    func=mybir.ActivationFunctionType.Exp,
                    scale=-1.0,
                )

                pav = ps_av.tile([C, G, 128], F32)
                for w in range(G):
                    for h in range(H):
                        nc.tensor.matmul(
                            out=pav[h * HD : (h + 1) * HD, w, 0:N],
                            lhsT=v4[:, w, h * HD : (h + 1) * HD],
                            rhs=expT[:, w, h * N : (h + 1) * N],
                        )
                avT = avT_pool.tile([C + 1, G, N], BF16)
                if g < 2:
                    nc.gpsimd.memset(avT[C : C + 1, :, :], 1.0)
                nc.vector.tensor_tensor(
                    out=avT[0:C, :, :],
                    in0=pav[:, :, 0:N],
                    in1=r2,
                    op=mybir.AluOpType.mult,
                )
                psy = ps_y.tile([C, G * N], F32)
                nc.tensor.matmul(out=psy, lhsT=pw2_sb, rhs=avT)
                if g % 2 == 0:
                    ystage = ystage_pool.tile([C, 2, G * N], BF16)
                nc.vector.tensor_copy(ystage[:, g % 2, :], psy)
                if g % 2 == 1:
                    blk = g // 2
                    nc.sync.dma_start(
                        out=out[:, blk * YB * N : (blk + 1) * YB * N],
                        in_=ystage,
                    )

            for it in range(NG + 1):
                if it < NG:
                    emit_head(it)
                if it > 0:
                    emit_tail(it - 1)
                if it < NG:
                    emit_logits(it)

    if split_waits:
        _split_sync_waits(nc)
    return nc


def _get_program():
    global _PROGRAM
    if _PROGRAM is None:
        _PROGRAM = _build_program()
    return _PROGRAM


# ------------------------------------------------------------------- kernel
def _core_instance_bidx(c):
    """B_ indices for core c's 128 window-instances, in device order."""
    w = np.arange(NI)
    return 512 * (w % 2) + NJ * c + (w // 2)


def _prepare_in_maps(x, y, mask, qkv_w, rpb_table, proj_w, proj_b):
    x = np.asarray(x, dtype=np.float32)
    y = np.asarray(y, dtype=np.float32)
    mask = np.asarray(mask, dtype=np.float32)
    qkv_w = np.asarray(qkv_w, dtype=np.float64)
    rpb_table = np.asarray(rpb_table, dtype=np.float32)
    proj_w = np.asarray(proj_w, dtype=np.float32)
    proj_b = np.asarray(proj_b, dtype=np.float32)

    scale = float(HD) ** -0.5

    # additive mask+bias table: amb[tk, j, h*98+tq]
    bias = rpb_table[REL_IDX.reshape(-1)].reshape(N, N, H).transpose(2, 0, 1)
    amb_all = mask[:, None, :, :] + bias[None, :, :, :]   # (NW, H, tq, tk)
    amb_t = np.ascontiguousarray(amb_all.transpose(3, 0, 1, 2)).reshape(
        N, NW, HB
    )

    # per-head composite: zw[:, h, :] = scale * Wq_h^T @ Wk_h  (96x96)
    zw_h = np.empty((C, H, C), dtype=np.float64)
    for h in range(H):
        wq_h = qkv_w[h * HD : (h + 1) * HD, :]            # (32, 96)
        wk_h = qkv_w[C + h * HD : C + (h + 1) * HD, :]    # (32, 96)
        zw_h[:, h, :] = scale * (wq_h.T @ wk_h)
    zw_h = zw_h.astype(NPBF16)

    wv_h = np.ascontiguousarray(qkv_w[2 * C : 3 * C].astype(np.float32).T
                                ).astype(NPBF16)
    pw2_h = np.empty((C + 1, C), dtype=np.float32)
    pw2_h[0:C] = proj_w.T
    pw2_h[C] = proj_b
    pw2_h = pw2_h.astype(NPBF16)
    ident_h = np.eye(N, dtype=np.float32).astype(NPBF16)

    in_maps = []
    bidx = []
    for c in range(NCORES):
        bi = _core_instance_bidx(c)
        bidx.append(bi)
        xc = x[bi].reshape(T, C)
        yc = y[bi].reshape(T, C)
        amb_c = np.ascontiguousarray(
            amb_t[:, NJ * c : NJ * (c + 1), :]
        ).astype(NPBF16)
        in_maps.append(
            {
                "xT": np.ascontiguousarray(xc.T).astype(NPBF16),
                "yT": np.ascontiguousarray(yc.T).astype(NPBF16),
                "amb": amb_c,
                "zw": zw_h,
                "wv": wv_h,
                "ident": ident_h,
                "pw2": pw2_h,
            }
        )
    return in_maps, bidx


def kernel(x, y, mask, qkv_w, rpb_table, proj_w, proj_b):
    in_maps, bidx = _prepare_in_maps(
        x, y, mask, qkv_w, rpb_table, proj_w, proj_b
    )
    nc = _get_program()
    res = run_bass_kernel_spmd(nc, in_maps, list(range(NCORES)))

    out_full = np.empty((BWIN, N, C), dtype=np.float32)
    for c in range(NCORES):
        yt_o = np.asarray(res.results[c]["yT_out"]).astype(np.float32)
        out_full[bidx[c]] = yt_o.T.reshape(NI, N, C)
    return out_full


# revision 14
# speedup vs baseline: 1.0600x; 1.0600x over previous
"""Bass/Trainium2 kernel for nn_CrossWindowAttention3D (8-core SPMD).

Strategy (hardcoded for shapes B_=1024, N=98, C=96, H=3, NW=512):
- Shard 1024 window-instances over 8 cores: core c owns distinct windows
  [64c, 64c+64) for both batch replicas, interleaved (b0,j),(b1,j) so the
  mask+bias table for window j is loaded once per pair.
- Host precomputes channel-major bf16 transposes of x/y, per-head composite
  matrices M_h = scale * Wq_h^T Wk_h (so logits_h = x^T (M_h y) and no
  separate k projection is needed), the additive mask+bias table
  amb[tk, j, h*98+tq] = mask[j, tq, tk] + bias[h, tq, tk], and the proj
  weight with a trailing bias row (pw2 [97, 96], row 96 = proj_b).
- Device, per 4-window group (2 pairs x 2 batch replicas):
  * zt = M_h y for 4 windows (3 full matmuls, 392 cols each)
  * per window: mask+bias accumulated into logits PSUM via an
    identity-stationary matmul, then logits = x^T zt on top (PSUM acc);
    the same x stationary also produces token-major v (v = x^T Wv).
  * one quad-window exp on ACT ([98, 4, 294] spanning 4 PSUM banks)
  * softmax denominators via narrow ones-stationary matmuls (out 32-row
    col-groups run concurrently in the PE array), reciprocal via the
    1-op DVE reciprocal_approx_fast, av via 12 narrow matmuls, normalize
    with one DVE multiply into a [97, 392] tile whose last row is 1.0,
    projection (+bias via the ones row) as one 392-col matmul, output
    staged bf16 and DMA'd channel-major; host transposes/casts to f32.
- PSUM budget: shared 4-bank region (zt slots then logits), pv 1, pd 1,
  pav 1, psy 1 = 8 banks. Emission is software-pipelined: group g+1's zt
  and logits interleave with group g's denominator/av/proj tail.
"""

import sys

sys.path.insert(0, "/opt/trn_rl_repo")

import numpy as np
import ml_dtypes

import concourse.bass as bass
import concourse.tile as tile
from concourse import mybir
from concourse.vector_clock import ScopedClock
from concourse.bass_utils import run_bass_kernel_spmd

BF16 = mybir.dt.bfloat16
F32 = mybir.dt.float32
NPBF16 = ml_dtypes.bfloat16

WS = (2, 7, 7)
N = 98            # tokens per window
C = 96            # embed dim
H = 3             # heads
HD = 32           # head dim
NW = 512          # distinct windows
BWIN = 1024       # window-instances total
NCORES = 8
NI = 128          # instances per core
NJ = 64           # distinct windows per core
T = NI * N        # tokens per core = 12544
HB = H * N        # 294
G = 4             # windows per group
NG = NI // G      # 32 groups


# ---------------------------------------------------------------- tile patch
def _patch_tile_tail_drain():
    """This neuronxcc build rejects >1 sync wait on CTRL-class (Drain)
    instructions; split the TileContext tail-drain waits across NOPs."""
    if getattr(tile.TileContext, "_drain_patch_applied", False):
        return

    def _drain_and_barrier_split(self, tick_clock, wait_clock):
        nc = self.nc
        carrier = nc.sync.nop(nofuse=True)
        wait_clock.add_sem_waits(
            carrier.ins, ScopedClock({None: tick_clock.global_clock})
        )
        si = carrier.ins.sync_info
        waits = list(si.on_wait or []) if si is not None else []
        if len(waits) > 1:
            si.on_wait = waits[:1]
            for w in waits[1:]:
                extra = nc.sync.nop(nofuse=True)
                esi = extra.ins.sync_info
                if esi is None:
                    extra.ins.sync_info = mybir.SyncInfo(
                        on_wait=[w], on_update=[]
                    )
                else:
                    esi.on_wait = list(esi.on_wait or []) + [w]
        nc.sync.drain()
        nc.all_engine_barrier()
        assert self.sems is not None
        popped = nc._tile_sem_poison_stack.pop()
        assert popped is self._sem_poison
        nc.clear_and_free_semaphores(list(self.sems.allocated().values()))
        nc.all_engine_barrier()

    tile.TileContext._drain_and_barrier = _drain_and_barrier_split
    tile.TileContext._drain_patch_applied = True


def _split_sync_waits(nc, max_waits=1):
    """This neuronxcc build accepts at most one sync wait per instruction.
    Hoist excess waits onto same-engine NOPs inserted just before the
    instruction (the sequencer blocks on them in order; AND-semantics of
    multiple waits is preserved)."""
    ctr = 0
    for bb in nc.main_func.blocks:
        new_list = []
        changed = False
        for inst in bb.instructions:
            si = inst.sync_info
            waits = list(si.on_wait or []) if si is not None else []
            if len(waits) > max_waits:
                si.on_wait = waits[: max_waits]
                for w in waits[max_waits:]:
                    nop = mybir.InstNoOp(
                        name=f"I-waitsplit-{ctr}", ins=[], outs=[]
                    )
                    ctr += 1
                    nop.engine = inst.engine
                    nop.sync_info = mybir.SyncInfo(on_wait=[w], on_update=[])
                    new_list.append(nop)
                changed = True
            new_list.append(inst)
        if changed:
            bb.instructions = new_list


# ------------------------------------------------------------- host helpers
def _relative_position_index():
    ws = WS
    coords = np.stack(
        np.meshgrid(
            np.arange(ws[0]), np.arange(ws[1]), np.arange(ws[2]), indexing="ij"
        )
    )
    cf = coords.reshape(3, -1)
    rel = cf[:, :, None] - cf[:, None, :]
    rel = rel.transpose(1, 2, 0).astype(np.int64)
    rel[..., 0] += ws[0] - 1
    rel[..., 1] += ws[1] - 1
    rel[..., 2] += ws[2] - 1
    rel[..., 0] *= (2 * ws[1] - 1) * (2 * ws[2] - 1)
    rel[..., 1] *= 2 * ws[2] - 1
    return rel.sum(-1)  # (N, N)


REL_IDX = _relative_position_index()


# ------------------------------------------------------------ device program
_PROGRAM = None

# tiling knobs
XCH = 32          # instances per x/y SBUF chunk (4 chunks)
ACH = 16          # distinct windows per amb SBUF chunk
YB = 8            # windows per output staging buffer / DMA


def _build_program(split_waits=True):
    _patch_tile_tail_drain()
    nc = bass.Bass()

    xT = nc.declare_dram_parameter("xT", [C, T], BF16, isOutput=False)
    yT = nc.declare_dram_parameter("yT", [C, T], BF16, isOutput=False)
    amb = nc.declare_dram_parameter("amb", [N, NJ, HB], BF16, isOutput=False)
    zw = nc.declare_dram_parameter("zw", [C, H, C], BF16, isOutput=False)
    wv = nc.declare_dram_parameter("wv", [C, C], BF16, isOutput=False)
    ident = nc.declare_dram_parameter("ident", [N, N], BF16, isOutput=False)
    pw2 = nc.declare_dram_parameter("pw2", [C + 1, C], BF16, isOutput=False)
    out = nc.declare_dram_parameter("yT_out", [C, T], BF16, isOutput=True)

    from contextlib import ExitStack

    with tile.TileContext(nc) as tc:
        with ExitStack() as ctx:
            singles = ctx.enter_context(tc.tile_pool(name="singles", bufs=1))
            xt_pool = ctx.enter_context(tc.tile_pool(name="xt", bufs=2))
            yt_pool = ctx.enter_context(tc.tile_pool(name="yt", bufs=2))
            amb_pool = ctx.enter_context(tc.tile_pool(name="amb", bufs=2))
            ztq_pool = ctx.enter_context(tc.tile_pool(name="ztq", bufs=2))
            expT_pool = ctx.enter_context(tc.tile_pool(name="expT", bufs=2))
            v4_pool = ctx.enter_context(tc.tile_pool(name="v4", bufs=2))
            r2_pool = ctx.enter_context(tc.tile_pool(name="r2", bufs=2))
            avT_pool = ctx.enter_context(tc.tile_pool(name="avT", bufs=2))
            ystage_pool = ctx.enter_context(
                tc.tile_pool(name="ystage", bufs=2)
            )
            # PSUM (8 banks): per-pair logits tiles [98,2,512] bufs=2 (4),
            # zt [96,3,512] (3), and ONE bank time-shared by pv/pd/psy via
            # a common tag ring (their lifetimes are strictly ordered).
            ps_lg = ctx.enter_context(
                tc.tile_pool(name="ps_lg", bufs=2, space="PSUM")
            )
            ps_zt = ctx.enter_context(
                tc.tile_pool(name="ps_zt", bufs=1, space="PSUM")
            )
            ps_aux = ctx.enter_context(
                tc.tile_pool(name="ps_aux", bufs=1, space="PSUM")
            )

            zw_sb = singles.tile([C, H, C], BF16)
            nc.sync.dma_start(out=zw_sb, in_=zw[:, :, :])
            wv_sb = singles.tile([C, C], BF16)
            nc.sync.dma_start(out=wv_sb, in_=wv[:, :])
            id_sb = singles.tile([N, N], BF16)
            nc.sync.dma_start(out=id_sb, in_=ident[:, :])
            pw2_sb = singles.tile([C + 1, C], BF16)
            nc.sync.dma_start(out=pw2_sb, in_=pw2[:, :])
            ones_sb = singles.tile([N, HD], BF16)
            nc.vector.memset(ones_sb, 1.0)

            # per-group state carried across the pipelined loop
            st = {}
            xt_ch = yt_ch = amb_ch = None
            ystage = None

            def emit_head_a(g):
                """chunk loads + zt matmuls for heads 0..1 of group g."""
                nonlocal xt_ch, yt_ch, amb_ch
                w0 = G * g
                if w0 % XCH == 0:
                    ch = w0 // XCH
                    xt_ch = xt_pool.tile([C, XCH * N], BF16)
                    nc.sync.dma_start(
                        out=xt_ch,
                        in_=xT[:, ch * XCH * N : (ch + 1) * XCH * N],
                    )
                    yt_ch = yt_pool.tile([C, XCH * N], BF16)
                    nc.sync.dma_start(
                        out=yt_ch,
                        in_=yT[:, ch * XCH * N : (ch + 1) * XCH * N],
                    )
                # distinct windows for group g are 2g, 2g+1
                if (2 * g) % ACH == 0:
                    ak = (2 * g) // ACH
                    amb_ch = amb_pool.tile([N, ACH, HB], BF16)
                    nc.sync.dma_start(
                        out=amb_ch, in_=amb[:, ak * ACH : (ak + 1) * ACH, :]
                    )

                pz = ps_zt.tile([C, H, 512], F32)
                goff = (w0 % XCH) * N
                for h in range(2):
                    nc.tensor.matmul(
                        out=pz[:, h, 0 : G * N],
                        lhsT=zw_sb[:, h, :],
                        rhs=yt_ch[:, goff : goff + G * N],
                    )
                st[g] = {
                    "pz": pz,
                    "goff": goff,
                    "amb_ch": amb_ch,
                    "xt_ch": xt_ch,
                    "yt_ch": yt_ch,
                }

            def emit_head_b(g):
                """zt matmul head 2 + PSUM->SBUF cast for group g."""
                sg = st[g]
                pz = sg.pop("pz")
                goff = sg["goff"]
                nc.tensor.matmul(
                    out=pz[:, 2, 0 : G * N],
                    lhsT=zw_sb[:, 2, :],
                    rhs=sg.pop("yt_ch")[:, goff : goff + G * N],
                )
                ztq = ztq_pool.tile([C, H, G * N], BF16)
                nc.vector.tensor_copy(ztq, pz[:, :, 0 : G * N])
                sg["ztq"] = ztq

            def emit_logits(g):
                """mask+bias accumulate, logits, v, exp, vcopy for group g."""
                sg = st[g]
                ztq = sg["ztq"]
                goff = sg["goff"]
                a_ch = sg["amb_ch"]
                x_ch = sg["xt_ch"]
                pv = ps_aux.tile([N, G, 128], F32, name="pv", tag="aux")
                expT = expT_pool.tile([N, G, HB], BF16)
                for pr in range(2):
                    j = 2 * g + pr           # distinct window
                    aj = j % ACH
                    Rp = ps_lg.tile([N, 2, 512], F32, name="Rp")
                    # mask+bias into PSUM (identity stationary, shared LDW)
                    for k in range(2):
                        nc.tensor.matmul(
                            out=Rp[:, k, 0:HB],
                            lhsT=id_sb,
                            rhs=a_ch[:, aj, :],
                            start=True,
                            stop=False,
                        )
                    for k in range(2):
                        w = 2 * pr + k       # window slot in group
                        col = goff + w * N
                        nc.tensor.matmul(
                            out=Rp[:, k, 0:HB],
                            lhsT=x_ch[:, col : col + N],
                            rhs=ztq[:, :, w * N : (w + 1) * N],
                            start=False,
                            stop=True,
                        )
                    nc.scalar.activation(
                        out=expT[:, 2 * pr : 2 * pr + 2, :],
                        in_=Rp[:, :, 0:HB],
                        func=mybir.ActivationFunctionType.Exp,
                    )
                for w in range(G):
                    col = goff + w * N
                    nc.tensor.matmul(
                        out=pv[:, w, 0:C],
                        lhsT=x_ch[:, col : col + N],
                        rhs=wv_sb,
                    )
                v4 = v4_pool.tile([N, G, C], BF16)
                nc.vector.tensor_copy(v4, pv[:, :, 0:C])
                sg["expT"] = expT
                sg["v4"] = v4

            def emit_tail_a(g):
                """denominators + Ln for group g."""
                sg = st[g]
                expT = sg["expT"]
                pd = ps_aux.tile([C, G, N], F32, name="pd", tag="aux")
                for h in range(H):
                    nc.tensor.matmul(
                        out=pd[h * HD : (h + 1) * HD, :, :],
                        lhsT=ones_sb,
                        rhs=expT[:, :, h * N : (h + 1) * N],
                    )
                # 1/d = exp(-ln(d)); Ln+Exp share one ACT table set
                t_ln = r2_pool.tile([C, G, N], F32, name="t_ln", tag="tl")
                nc.scalar.activation(
                    out=t_ln,
                    in_=pd,
                    func=mybir.ActivationFunctionType.Ln,
                )
                sg["t_ln"] = t_ln

            def emit_tail_b(g):
                """av, reciprocal, normalize, proj, out for group g."""
                nonlocal ystage
                sg = st.pop(g)
                expT = sg["expT"]
                v4 = sg["v4"]
                pav = ps_aux.tile([C, G, 128], F32, name="pav", tag="aux")
                for w in range(G):
                    for h in range(H):
                        nc.tensor.matmul(
                            out=pav[h * HD : (h + 1) * HD, w, 0:N],
                            lhsT=v4[:, w, h * HD : (h + 1) * HD],
                            rhs=expT[:, w, h * N : (h + 1) * N],
                        )
                r2 = r2_pool.tile([C, G, N], F32, name="r2", tag="r2")
                nc.scalar.activation(
                    out=r2,
                    in_=sg.pop("t_ln"),
                    func=mybir.ActivationFunctionType.Exp,
                    scale=-1.0,
                )
                avT = avT_pool.tile([C + 1, G, N], BF16)
                if g < 2:
                    nc.gpsimd.memset(avT[C : C + 1, :, :], 1.0)
                nc.vector.tensor_tensor(
                    out=avT[0:C, :, :],
                    in0=pav[:, :, 0:N],
                    in1=r2,
                    op=mybir.AluOpType.mult,
                )
                psy = ps_aux.tile([C, G * N], F32, name="psy", tag="aux")
                nc.tensor.matmul(out=psy, lhsT=pw2_sb, rhs=avT)
                if g % 2 == 0:
                    ystage = ystage_pool.tile([C, 2, G * N], BF16)
                nc.vector.tensor_copy(ystage[:, g % 2, :], psy)
                if g % 2 == 1:
                    blk = g // 2
                    nc.sync.dma_start(
                        out=out[:, blk * YB * N : (blk + 1) * YB * N],
                        in_=ystage,
                    )

            emit_head_a(0)
            emit_head_b(0)
            for it in range(NG):
                emit_logits(it)
                if it + 1 < NG:
                    emit_head_a(it + 1)
                emit_tail_a(it)
                if it + 1 < NG:
                    emit_head_b(it + 1)
                emit_tail_b(it)

    if split_waits:
        _split_sync_waits(nc)
    return nc


def _get_program():
    global _PROGRAM
    if _PROGRAM is None:
        _PROGRAM = _build_program()
    return _PROGRAM


# ------------------------------------------------------------------- kernel
def _core_instance_bidx(c):
    """B_ indices for core c's 128 window-instances, in device order."""
    w = np.arange(NI)
    return 512 * (w % 2) + NJ * c + (w // 2)


def _prepare_in_maps(x, y, mask, qkv_w, rpb_table, proj_w, proj_b):
    x = np.asarray(x, dtype=np.float32)
    y = np.asarray(y, dtype=np.float32)
    mask = np.asarray(mask, dtype=np.float32)
    qkv_w = np.asarray(qkv_w, dtype=np.float64)
    rpb_table = np.asarray(rpb_table, dtype=np.float32)
    proj_w = np.asarray(proj_w, dtype=np.float32)
    proj_b = np.asarray(proj_b, dtype=np.float32)

    scale = float(HD) ** -0.5

    # additive mask+bias table: amb[tk, j, h*98+tq]
    bias = rpb_table[REL_IDX.reshape(-1)].reshape(N, N, H).transpose(2, 0, 1)
    amb_all = mask[:, None, :, :] + bias[None, :, :, :]   # (NW, H, tq, tk)
    amb_t = np.ascontiguousarray(amb_all.transpose(3, 0, 1, 2)).reshape(
        N, NW, HB
    )

    # per-head composite: zw[:, h, :] = scale * Wq_h^T @ Wk_h  (96x96)
    zw_h = np.empty((C, H, C), dtype=np.float64)
    for h in range(H):
        wq_h = qkv_w[h * HD : (h + 1) * HD, :]            # (32, 96)
        wk_h = qkv_w[C + h * HD : C + (h + 1) * HD, :]    # (32, 96)
        zw_h[:, h, :] = scale * (wq_h.T @ wk_h)
    zw_h = zw_h.astype(NPBF16)

    wv_h = np.ascontiguousarray(qkv_w[2 * C : 3 * C].astype(np.float32).T
                                ).astype(NPBF16)
    pw2_h = np.empty((C + 1, C), dtype=np.float32)
    pw2_h[0:C] = proj_w.T
    pw2_h[C] = proj_b
    pw2_h = pw2_h.astype(NPBF16)
    ident_h = np.eye(N, dtype=np.float32).astype(NPBF16)

    in_maps = []
    bidx = []
    for c in range(NCORES):
        bi = _core_instance_bidx(c)
        bidx.append(bi)
        xc = x[bi].reshape(T, C)
        yc = y[bi].reshape(T, C)
        amb_c = np.ascontiguousarray(
            amb_t[:, NJ * c : NJ * (c + 1), :]
        ).astype(NPBF16)
        in_maps.append(
            {
                "xT": np.ascontiguousarray(xc.T).astype(NPBF16),
                "yT": np.ascontiguousarray(yc.T).astype(NPBF16),
                "amb": amb_c,
                "zw": zw_h,
                "wv": wv_h,
                "ident": ident_h,
                "pw2": pw2_h,
            }
        )
    return in_maps, bidx


def kernel(x, y, mask, qkv_w, rpb_table, proj_w, proj_b):
    in_maps, bidx = _prepare_in_maps(
        x, y, mask, qkv_w, rpb_table, proj_w, proj_b
    )
    nc = _get_program()
    res = run_bass_kernel_spmd(nc, in_maps, list(range(NCORES)))

    out_full = np.empty((BWIN, N, C), dtype=np.float32)
    for c in range(NCORES):
        yt_o = np.asarray(res.results[c]["yT_out"]).astype(np.float32)
        out_full[bidx[c]] = yt_o.T.reshape(NI, N, C)
    return out_full
